# revision 22
# baseline (speedup 1.0000x reference)
"""BiCGSTAB (4 fixed iterations, 7-point stencil) on 8 Trainium2 NeuronCores.

Problem: x,b,ref: [2,256,256,256] f32, center: [1,256,256,1] f32.
reference() runs 4 BiCGSTAB iterations of A.u where A is the 7-point stencil
  S(u)[b,h,w,z] = center[h,w]*u - u[w-1] - u[w+1] - u[h-1] - u[h+1] - u[z-1] - u[z+1]
with zero Dirichlet boundaries, and global (per-batch) dot products.

Sharding: core c in 0..7 handles batch b=c//4 and H-slab [64*(c%4), 64*(c%4)+64).
Dot products become 4-rank AllReduces in groups [[0..3],[4..7]]. H-halo planes
are exchanged via AllGather within the group + indirect-DMA ghost-row gathers
(edge cores index a zeroed row range, implementing the Dirichlet boundary).

v2 design (vs f32 baseline):
- Fields r0,r,p,v,s,t stored bf16 (halved HBM traffic, 2x DVE rate); x kept
  f32. Dot products accumulate f32.
- r0 is SBUF-resident for the whole kernel (no DRAM reads for dot products).
- The ENTIRE 7-point stencil runs on the TensorEngine: W-shifts via tridiag
  matmul, H-shifts via +-Z offsets and Z-shifts via +-1 offsets on the
  flattened (h,z) moving operand, accumulated in PSUM. Two tiny DVE ops fix
  the z-wraparound columns. DVE only does the final (center*u - psum) combine
  (center*u on the Act engine).
- s = r - alpha*v is computed on the fly inside the t-pass (windows), so the
  separate P2 sweep and one full s round-trip disappear.
- Dots run partly on Pool (gpsimd) to offload DVE.
- P0 (f32 x windows) uses float32r matmuls (1 cycle/row vs 4 for plain f32).
"""
import numpy as np

import concourse.bacc as bacc
import concourse.bass as bass
import concourse.bass_isa as bass_isa
import concourse.mybir as mybir
import concourse.tile as tile

F32 = mybir.dt.float32
F16 = mybir.dt.float16
BF16 = mybir.dt.bfloat16
I32 = mybir.dt.int32

N_CORES = 8
GROUP = 4  # cores per batch group
EPS = 1e-6

KH = 8    # h planes per stencil block (fp16 passes)
KH0 = 4   # h planes per block in P0 (f32 windows)
# Stencil outputs are computed as lam*S(u) (lam folded into cen and the shift
# matrices, both exact in fp16) so v and t stay inside fp16 range; the scalar
# coefficients compensate (alpha*ILAM etc).
LAM = 1.0 / 256.0
ILAM = 256.0


def build_program(HC=64, W=256, Z=256, ITERS=4, collectives=True,
                  maxph=99, twin_reps=0):
    """Build the per-core SPMD Bass program. HC = H planes per core.

    collectives=False builds a single-core timing twin (collective_compute
    calls skipped; numerics wrong) usable for wall-clock delta timing.
    """
    assert W == 256 and Z == 256 and HC % KH == 0 and HC % KH0 == 0
    NB = HC // KH
    NB0 = HC // KH0
    RG = [list(range(GROUP)), list(range(GROUP, 2 * GROUP))]
    # halo buffer row layouts (rows = field*2W + side*W + w_global)
    ZR_A = GROUP * 4 * W   # zero-row base in haloA_out (2 fields)
    ZR_B = GROUP * 2 * W   # zero-row base in haloB/C_out (1 field)

    twin = twin_reps > 0
    assert not (twin and collectives), "twin loop cannot contain collectives"
    nc = bacc.Bacc("TRN2", target_bir_lowering=False, debug=False,
                   num_devices=N_CORES)

    if twin:
        x_in = nc.dram_tensor("xin_t", [W, HC, Z], F32)
        b_in = nc.dram_tensor("bin_t", [W, HC, Z], F32)
        x_out = nc.dram_tensor("xout_t", [W, HC, Z], F32)
        dummy_out = nc.dram_tensor("dummy_o", [1, 8], F32, kind="ExternalOutput")
    else:
        x_in = nc.dram_tensor("x", [W, HC, Z], F32, kind="ExternalInput")
        b_in = nc.dram_tensor("bb", [W, HC, Z], F32, kind="ExternalInput")
        x_out = nc.dram_tensor("xout", [W, HC, Z], F32, kind="ExternalOutput")
    cen_in = nc.dram_tensor("cen", [W, HC], F32, kind="ExternalInput")
    matsb_in = nc.dram_tensor("matsb", [128, 512], F16, kind="ExternalInput")
    idxA_in = nc.dram_tensor("idxA", [W, 4], I32, kind="ExternalInput")
    idxB_in = nc.dram_tensor("idxB", [W, 2], I32, kind="ExternalInput")

    with tile.TileContext(nc) as tc:
        with (
            tc.tile_pool(name="sb", bufs=2) as sb,
            tc.tile_pool(name="ps", bufs=8, space="PSUM") as ps,
            tc.tile_pool(name="dr", bufs=1, space="DRAM") as dr,
        ):
            _cnt = [0]

            def _nm(pfx):
                _cnt[0] += 1
                return f"{pfx}{_cnt[0]}"

            # ---- persistent DRAM intermediates
            fld = {n: dr.tile([W, HC, Z], F16, tag=n, name=f"fld_{n}")
                   for n in ("r", "p", "v", "s", "t")}
            xw = dr.tile([W, HC, Z], F16, tag="xw", name="fld_xw")
            haloA_in = dr.tile([4 * W, Z], F16, tag="hAi")
            haloA_out = dr.tile([ZR_A + 128, Z], F16, tag="hAo")
            haloB_in = dr.tile([2 * W, Z], F16, tag="hBi")
            haloB_out = dr.tile([ZR_B + 128, Z], F16, tag="hBo")
            haloC_in = dr.tile([2 * W, Z], F32, tag="hCi")
            haloC_out = dr.tile([ZR_B + 128, Z], F32, tag="hCo")
            din = dr.tile([1, 8], F32, tag="din")
            dout = dr.tile([1, 8], F32, tag="dout")

            # ---- persistent SBUF constants
            cen_sb = []
            for wc in range(2):
                c = sb.tile([128, HC], F32, tag=f"cen{wc}", bufs=1)
                nc.sync.dma_start(out=c[:], in_=cen_in[wc * 128:(wc + 1) * 128, :])
                cen_sb.append(c)
            matsb_sb = sb.tile([128, 512], F16, tag="matsb", bufs=1)
            nc.sync.dma_start(out=matsb_sb[:], in_=matsb_in[:, :])
            idxA_sb = []
            idxB_sb = []
            for wc in range(2):
                ia = sb.tile([128, 4], I32, tag=f"idxA{wc}", bufs=1)
                nc.sync.dma_start(out=ia[:], in_=idxA_in[wc * 128:(wc + 1) * 128, :])
                idxA_sb.append(ia)
                ib = sb.tile([128, 2], I32, tag=f"idxB{wc}", bufs=1)
                nc.sync.dma_start(out=ib[:], in_=idxB_in[wc * 128:(wc + 1) * 128, :])
                idxB_sb.append(ib)
            # resident r0 (bf16), [128, HC, Z] per W-chunk
            r0_sb = [sb.tile([128, HC, Z], F16, tag=f"r0_{wc}", bufs=1,
                             name=f"r0sb{wc}") for wc in range(2)]

            # zero tails + din
            ztb = sb.tile([128, Z], F16, tag="ghb", name="ztb")
            nc.vector.memset(ztb[:], 0.0)
            nc.sync.dma_start(out=haloA_out[ZR_A:ZR_A + 128, :], in_=ztb[:])
            nc.sync.dma_start(out=haloB_out[ZR_B:ZR_B + 128, :], in_=ztb[:])
            ztf = sb.tile([128, Z], F32, tag="ghf", name="ztf")
            nc.vector.memset(ztf[:], 0.0)
            nc.sync.dma_start(out=haloC_out[ZR_B:ZR_B + 128, :], in_=ztf[:])
            z8 = sb.tile([1, 8], F32, tag="z8", bufs=1)
            nc.vector.memset(z8[:], 0.0)
            nc.sync.dma_start(out=din[:, :], in_=z8[:])

            # matrix APs
            A_b, B01_b, B10_b, I_b = (matsb_sb[:, 0:128], matsb_sb[:, 128:256],
                                      matsb_sb[:, 256:384], matsb_sb[:, 384:512])

            # ---- helpers ------------------------------------------------
            def border_order(nb):
                e = [j for j in range(nb) if 0 < j < nb - 1]
                return e + ([0] if nb == 1 else [0, nb - 1])

            def load_window(field, wc, j, tag, kh, halo_out_t, idx_t, cols,
                            dt_):
                """[128, kh+2, Z] window of planes j*kh-1 .. j*kh+kh."""
                h0 = j * kh
                nb = HC // kh
                w0 = wc * 128
                win = sb.tile([128, kh + 2, Z], dt_, tag=tag, name=_nm("win"))
                lo_g = (j == 0)
                hi_g = (j == nb - 1)
                a = 0 if lo_g else h0 - 1
                bnd = HC if hi_g else h0 + kh + 1
                po = 1 if lo_g else 0
                nc.sync.dma_start(
                    out=win[:, po:po + (bnd - a), :],
                    in_=field[w0:w0 + 128, a:bnd, :])
                if lo_g:
                    nc.gpsimd.indirect_dma_start(
                        out=win[:, 0, :], out_offset=None, in_=halo_out_t[:, :],
                        in_offset=bass.IndirectOffsetOnAxis(
                            ap=idx_t[wc][:, cols[0]:cols[0] + 1], axis=0))
                if hi_g:
                    nc.gpsimd.indirect_dma_start(
                        out=win[:, kh + 1, :], out_offset=None,
                        in_=halo_out_t[:, :],
                        in_offset=bass.IndirectOffsetOnAxis(
                            ap=idx_t[wc][:, cols[1]:cols[1] + 1], axis=0))
                return win

            def stencil_tile(wins, wc, j, kh, act_wins=None, au_dt=F16):
                """vt = S(u) for chunk wc, block j, from (win0, win1).

                wins feed the PE matmuls (bf16); act_wins (default: same)
                feed the center-coefficient muls on the Act engine.
                """
                h0 = j * kh
                win = wins[wc]
                awin = (act_wins or wins)[wc]
                other = wins[1 - wc]
                wf_m = win[:].rearrange("p h z -> p (h z)")
                of_m = other[:].rearrange("p h z -> p (h z)")
                A_, I_ = A_b, I_b
                Bm = B01_b if wc == 0 else B10_b
                au = sb.tile([128, kh, Z], au_dt, tag="au", name=_nm("au"))
                for j1 in range(kh):
                    h = h0 + j1
                    nc.scalar.mul(out=au[:, j1, :], in_=awin[:, j1 + 1, :],
                                  mul=cen_sb[wc][:, h:h + 1])
                auf = au[:].rearrange("p h z -> p (h z)")
                vt = sb.tile([128, kh, Z], F16, tag=f"o{wc}", name=_nm("vt"))
                vf = vt[:].rearrange("p h z -> p (h z)")
                for q in range(kh * Z // 512):
                    c0, c1 = q * 512, (q + 1) * 512
                    ceng = nc.vector
                    pt = ps.tile([128, 512], F32, tag="pt", name=_nm("pt"))
                    nc.tensor.matmul(out=pt[:], lhsT=A_,
                                     rhs=wf_m[:, Z + c0:Z + c1],
                                     start=True, stop=False)
                    nc.tensor.matmul(out=pt[:], lhsT=I_,
                                     rhs=wf_m[:, c0:c1],
                                     start=False, stop=False)
                    nc.tensor.matmul(out=pt[:], lhsT=I_,
                                     rhs=wf_m[:, 2 * Z + c0:2 * Z + c1],
                                     start=False, stop=False)
                    nc.tensor.matmul(out=pt[:], lhsT=I_,
                                     rhs=wf_m[:, Z + c0 - 1:Z + c1 - 1],
                                     start=False, stop=False)
                    nc.tensor.matmul(out=pt[:], lhsT=I_,
                                     rhs=wf_m[:, Z + c0 + 1:Z + c1 + 1],
                                     start=False, stop=False)
                    nc.tensor.matmul(out=pt[:], lhsT=Bm,
                                     rhs=of_m[:, Z + c0:Z + c1],
                                     start=False, stop=True)
                    ceng.tensor_tensor(out=vf[:, c0:c1], in0=auf[:, c0:c1],
                                       in1=pt[:],
                                       op=mybir.AluOpType.subtract)
                # undo z-shift wraparound at z=0 / z=Z-1 (scaled by lam)
                nc.vector.scalar_tensor_tensor(
                    out=vt[:, :, 0:1], in0=win[:, 0:kh, Z - 1:Z],
                    scalar=LAM, in1=vt[:, :, 0:1],
                    op0=mybir.AluOpType.mult, op1=mybir.AluOpType.add)
                nc.vector.scalar_tensor_tensor(
                    out=vt[:, :, Z - 1:Z], in0=win[:, 2:kh + 2, 0:1],
                    scalar=LAM, in1=vt[:, :, Z - 1:Z],
                    op0=mybir.AluOpType.mult, op1=mybir.AluOpType.add)
                return vt

            def ttr(in0, in1, acc_prev, tag="accA"):
                # dot-product partial: scr = LAM*in0*in1 (discarded), acc row
                # sums. LAM keeps the fp16 products in range; the reduced
                # dots are rescaled after the AllReduce. (DVE only:
                # TensorScalarPtr is not a Pool-engine opcode.)
                scr = sb.tile([128, KH, Z], F16, tag="au", name=_nm("scr"))
                sf = scr[:].rearrange("p h z -> p (h z)")
                n = in0.free_size()
                acc = sb.tile([128, 1], F32, tag=tag + "p", bufs=4,
                              name=_nm("acc"))
                nc.vector.scalar_tensor_tensor(
                    out=sf[:, 0:n], in0=in0, scalar=LAM, in1=in1,
                    op0=mybir.AluOpType.mult, op1=mybir.AluOpType.mult,
                    accum_out=acc[:])
                if acc_prev is None:
                    return acc
                tot = sb.tile([128, 1], F32, tag=tag, bufs=4, name=_nm("accs"))
                nc.vector.tensor_add(out=tot[:], in0=acc_prev[:], in1=acc[:])
                return tot

            def finish_dot(acc, col):
                red = sb.tile([128, 1], F32, tag="dscp", bufs=8,
                              name=_nm("red"))
                nc.gpsimd.partition_all_reduce(red[:], acc[:], channels=128,
                                               reduce_op=bass_isa.ReduceOp.add)
                nc.sync.dma_start(out=din[0:1, col:col + 1], in_=red[0:1, 0:1])

            def allreduce():
                if collectives:
                    nc.gpsimd.collective_compute(
                        "AllReduce", mybir.AluOpType.add, replica_groups=RG,
                        ins=[din[:, :].opt()], outs=[dout[:, :].opt()])
                else:
                    # twin: keep dsb finite (dout is never collective-written)
                    nc.sync.dma_start(out=dout[:, :], in_=din[:, :])
                dsb = sb.tile([1, 8], F32, tag="dsb", bufs=6, name=_nm("dsb"))
                nc.sync.dma_start(out=dsb[:], in_=dout[:, :])
                return dsb

            def allgather(halo_in_t, halo_out_t, zr):
                if not collectives:
                    return
                nc.gpsimd.collective_compute(
                    "AllGather", mybir.AluOpType.bypass, replica_groups=RG,
                    ins=[halo_in_t[:, :].opt()],
                    outs=[halo_out_t[0:zr, :].opt()])

            def stage_plane(src_plane, halo_in_t, f, side, wc):
                r0_ = f * 2 * W + side * W + wc * 128
                nc.sync.dma_start(out=halo_in_t[r0_:r0_ + 128, :],
                                  in_=src_plane)

            def s_tile():
                return sb.tile([1, 1], F32, tag="dsc", bufs=16, name=_nm("sc"))

            def s_recip_eps(a_ap):
                t_ = s_tile()
                nc.vector.tensor_scalar_add(out=t_[:], in0=a_ap, scalar1=EPS)
                r_ = s_tile()
                nc.vector.reciprocal(out=r_[:], in_=t_[:])
                return r_

            def s_mul(a_ap, b_ap):
                t_ = s_tile()
                nc.vector.tensor_tensor(out=t_[:], in0=a_ap, in1=b_ap,
                                        op=mybir.AluOpType.mult)
                return t_

            def s_sub(a_ap, b_ap):
                t_ = s_tile()
                nc.vector.tensor_tensor(out=t_[:], in0=a_ap, in1=b_ap,
                                        op=mybir.AluOpType.subtract)
                return t_

            def s_neg(a_ap):
                t_ = s_tile()
                nc.vector.tensor_scalar_mul(out=t_[:], in0=a_ap, scalar1=-1.0)
                return t_

            def s_scale(a_ap, imm):
                t_ = s_tile()
                nc.vector.tensor_scalar_mul(out=t_[:], in0=a_ap, scalar1=imm)
                return t_

            def bcast(a_ap):
                b_ = sb.tile([128, 1], F32, tag="bc", bufs=8, name=_nm("bc"))
                nc.gpsimd.partition_broadcast(b_[:], a_ap, channels=128)
                return b_

            def stt(out, in0, sc, in1, eng=None):
                """out = in0*sc + in1 (sc: [128,1] AP)."""
                (eng or nc.vector).scalar_tensor_tensor(
                    out=out, in0=in0, scalar=sc, in1=in1,
                    op0=mybir.AluOpType.mult, op1=mybir.AluOpType.add)

            def load_blk(field, wc, j, tag, dt_, kh=KH):
                t_ = sb.tile([128, kh, Z], dt_, tag=tag, name=_nm("blk"))
                h0 = j * kh
                w0 = wc * 128
                nc.sync.dma_start(out=t_[:],
                                  in_=field[w0:w0 + 128, h0:h0 + kh, :])
                return t_

            def store_blk(field, src, wc, j, kh=KH):
                h0 = j * kh
                w0 = wc * 128
                nc.sync.dma_start(out=field[w0:w0 + 128, h0:h0 + kh, :],
                                  in_=src)

            if twin:
                # init big inputs so the timing loop sees normal-range fp16
                # data (uninitialized DRAM decodes to NaN/denormals)
                zi = sb.tile([128, KH, Z], F32, tag="lx", name="zinit")
                nc.vector.memset(zi[:], 0.0)
                for wc in range(2):
                    for j in range(NB):
                        store_blk(x_in, zi[:], wc, j)
                        store_blk(b_in, zi[:], wc, j)

            border = border_order(NB)
            border0 = border_order(NB0)
            # halo-producing passes: edge blocks first
            ew_order = ([0, NB - 1] if NB > 1 else [0]) + list(range(1, NB - 1))

            # ================= P0: r0 = b - S(x); rho = <r0,r0> ===========
            from contextlib import ExitStack as _ES
            _loop = _ES()
            if twin:
                _loop.enter_context(tc.For_i(0, twin_reps, 1))

            # stage x boundary planes -> haloC, gather
            for wc in range(2):
                for side, h in ((0, 0), (1, HC - 1)):
                    g = sb.tile([128, Z], F32, tag="ghf", name=_nm("gh"))
                    nc.sync.dma_start(
                        out=g[:], in_=x_in[wc * 128:wc * 128 + 128, h, :])
                    stage_plane(g[:], haloC_in, 0, side, wc)
            allgather(haloC_in, haloC_out, ZR_B)

            acc = None
            rho_ap = None
            if maxph >= 2:
                for j in border0:
                    wins = (load_window(x_in, 0, j, "w0a", KH0, haloC_out,
                                        idxB_sb, (0, 1), F32),
                            load_window(x_in, 1, j, "w1a", KH0, haloC_out,
                                        idxB_sb, (0, 1), F32))
                    bwins = []
                    for wc in range(2):
                        bw = sb.tile([128, KH0 + 2, Z], F16,
                                     tag=f"w{wc}b", name=_nm("bw"))
                        nc.scalar.copy(out=bw[:], in_=wins[wc][:])
                        bwins.append(bw)
                    for wc in range(2):
                        vt = stencil_tile(tuple(bwins), wc, j, KH0,
                                          act_wins=wins, au_dt=F32)
                        bt = load_blk(b_in, wc, j, "lx", F32, kh=KH0)
                        h0 = j * KH0
                        r0sl = r0_sb[wc][:, h0:h0 + KH0, :]
                        nc.vector.scalar_tensor_tensor(
                            out=r0sl, in0=vt[:], scalar=-ILAM, in1=bt[:],
                            op0=mybir.AluOpType.mult, op1=mybir.AluOpType.add)
                        acc = ttr(r0sl, r0sl, acc)
                        store_blk(fld["p"], r0sl, wc, j, kh=KH0)
                        store_blk(fld["r"], r0sl, wc, j, kh=KH0)
                        if j == 0:
                            stage_plane(r0_sb[wc][:, 0, :], haloA_in, 0, 0, wc)
                            stage_plane(r0_sb[wc][:, 0, :], haloA_in, 1, 0, wc)
                        if j == NB0 - 1:
                            stage_plane(r0_sb[wc][:, HC - 1, :], haloA_in,
                                        0, 1, wc)
                            stage_plane(r0_sb[wc][:, HC - 1, :], haloA_in,
                                        1, 1, wc)
                finish_dot(acc, 0)
                dsb = allreduce()
                rho_ap = s_scale(dsb[0:1, 0:1], ILAM)[:]
                allgather(haloA_in, haloA_out, ZR_A)

            for it in range(ITERS if maxph >= 3 else 0):
                last = (it == ITERS - 1)
                x_src = x_in if it == 0 else xw
                x_dst = x_out if last else xw

                # ===== P1: v = S(p); d1 = <r0, v> =====
                acc = None
                for j in border:
                    wins = (load_window(fld["p"], 0, j, "w0a", KH, haloA_out,
                                        idxA_sb, (0, 1), F16),
                            load_window(fld["p"], 1, j, "w1a", KH, haloA_out,
                                        idxA_sb, (0, 1), F16))
                    for wc in range(2):
                        vt = stencil_tile(wins, wc, j, KH)
                        h0 = j * KH
                        acc = ttr(r0_sb[wc][:, h0:h0 + KH, :], vt[:], acc)
                        store_blk(fld["v"], vt[:], wc, j)
                        if j == 0:
                            stage_plane(vt[:, 0, :], haloB_in, 0, 0, wc)
                        if j == NB - 1:
                            stage_plane(vt[:, KH - 1, :], haloB_in, 0, 1, wc)
                finish_dot(acc, 0)
                dsb = allreduce()
                allgather(haloB_in, haloB_out, ZR_B)
                d1s = s_scale(dsb[0:1, 0:1], ILAM * ILAM)
                d1_ap = d1s[:]
                alpha = s_mul(rho_ap, s_recip_eps(d1_ap)[:])
                alpha_bc = bcast(alpha[:])
                nalpha_bc = bcast(s_scale(alpha[:], -ILAM)[:])
                if maxph < 4:
                    break

                # ===== P23: s = r - alpha*v (windows, on the fly);
                #            t = S(s); <t,s>, <t,t>, <r0,t> =====
                accA = accB = accC = None
                for j in border:
                    rwins = (load_window(fld["r"], 0, j, "w0a", KH, haloA_out,
                                         idxA_sb, (2, 3), F16),
                             load_window(fld["r"], 1, j, "w1a", KH, haloA_out,
                                         idxA_sb, (2, 3), F16))
                    vwins = (load_window(fld["v"], 0, j, "w0b", KH, haloB_out,
                                         idxB_sb, (0, 1), F16),
                             load_window(fld["v"], 1, j, "w1b", KH, haloB_out,
                                         idxB_sb, (0, 1), F16))
                    for wc in range(2):
                        # s window in-place over r window
                        nc.vector.scalar_tensor_tensor(
                            out=rwins[wc][:], in0=vwins[wc][:],
                            scalar=nalpha_bc[:], in1=rwins[wc][:],
                            op0=mybir.AluOpType.mult,
                            op1=mybir.AluOpType.add)
                    for wc in range(2):
                        tt = stencil_tile(rwins, wc, j, KH)
                        s_ctr = rwins[wc][:, 1:KH + 1, :]
                        accA = ttr(tt[:], s_ctr, accA, "accA")
                        accB = ttr(tt[:], tt[:], accB, "accB")
                        h0 = j * KH
                        if not last:
                            accC = ttr(r0_sb[wc][:, h0:h0 + KH, :],
                                       tt[:], accC, "accC")
                            store_blk(fld["t"], tt[:], wc, j)
                        store_blk(fld["s"], s_ctr, wc, j)
                finish_dot(accA, 0)
                finish_dot(accB, 1)
                if not last:
                    finish_dot(accC, 2)
                dsb = allreduce()
                ts_s = s_scale(dsb[0:1, 0:1], ILAM * ILAM)
                tt_s = s_scale(dsb[0:1, 1:2], ILAM * ILAM * ILAM)
                omega = s_mul(ts_s[:], s_recip_eps(tt_s[:])[:])
                omega_bc = bcast(omega[:])
                nomega_bc = bcast(s_scale(omega[:], -ILAM)[:])
                if not last:
                    # rho' = (rho - alpha*d1) - omega*<r0,t>
                    r0t_s = s_scale(dsb[0:1, 2:3], ILAM * ILAM)
                    rho_n = s_sub(s_sub(rho_ap, s_mul(alpha[:], d1_ap)[:])[:],
                                  s_mul(omega[:], r0t_s[:])[:])
                    beta = s_mul(
                        s_mul(rho_n[:], s_recip_eps(rho_ap)[:])[:],
                        s_mul(alpha[:], s_recip_eps(omega[:])[:])[:])
                    beta_bc = bcast(beta[:])
                    rho_ap = rho_n[:]
                if maxph < 5:
                    break

                # ===== P45: x += alpha*p + omega*s;
                #       r = s - omega*t;  p = r + beta*(p - omega*v) =====
                for wc in range(2):
                    for j in (ew_order if not last else list(range(NB))):
                        xt = load_blk(x_src, wc, j, "lx", F32 if it == 0 else F16)
                        pt_ = load_blk(fld["p"], wc, j, "lp", F16)
                        st = load_blk(fld["s"], wc, j, "ls", F16)
                        x1 = sb.tile([128, KH, Z], F16, tag="x1",
                                     name=_nm("x1"))
                        stt(x1[:], pt_[:], alpha_bc[:], xt[:])
                        x2 = sb.tile([128, KH, Z], F32 if last else F16, tag="lx",
                                     name=_nm("x2"))
                        stt(x2[:], st[:], omega_bc[:], x1[:])
                        store_blk(x_dst, x2[:], wc, j)
                        if not last:
                            tt_ = load_blk(fld["t"], wc, j, "lt", F16)
                            vt_ = load_blk(fld["v"], wc, j, "lv", F16)
                            rt = sb.tile([128, KH, Z], F16, tag="ls",
                                         name=_nm("rt"))
                            stt(rt[:], tt_[:], nomega_bc[:], st[:])
                            store_blk(fld["r"], rt[:], wc, j)
                            u = sb.tile([128, KH, Z], F16, tag="lv",
                                        name=_nm("u"))
                            stt(u[:], vt_[:], nomega_bc[:], pt_[:])
                            po = sb.tile([128, KH, Z], F16, tag="lp",
                                         name=_nm("po"))
                            stt(po[:], u[:], beta_bc[:], rt[:])
                            store_blk(fld["p"], po[:], wc, j)
                            if j == 0:
                                stage_plane(po[:, 0, :], haloA_in, 0, 0, wc)
                                stage_plane(rt[:, 0, :], haloA_in, 1, 0, wc)
                            if j == NB - 1:
                                stage_plane(po[:, KH - 1, :], haloA_in,
                                            0, 1, wc)
                                stage_plane(rt[:, KH - 1, :], haloA_in,
                                            1, 1, wc)
                if last:
                    break
                allgather(haloA_in, haloA_out, ZR_A)

            _loop.close()
            if twin:
                nc.sync.dma_start(out=dummy_out[:, :], in_=z8[:])

    nc.compile()
    return nc


# ---------------------------------------------------------------------------
# host-side wrapper
# ---------------------------------------------------------------------------
_CACHE = {}


def _shift_mats():
    """[A | B01 | B10 | I] as [128, 512], scaled by LAM (exact in fp16)."""
    lam = np.float32(1.0 / 256.0)
    A = np.zeros((128, 128), np.float32)
    for i in range(127):
        A[i, i + 1] = lam
        A[i + 1, i] = lam
    B01 = np.zeros((128, 128), np.float32)
    B01[0, 127] = lam
    B10 = np.zeros((128, 128), np.float32)
    B10[127, 0] = lam
    I = lam * np.eye(128, dtype=np.float32)
    return np.concatenate([A, B01, B10, I], axis=1)


def make_const_inputs(s, HC=64, W=256, twin=False):
    """Per-core constant inputs (core's slab index s within its group).

    twin=True points every ghost at the zeroed rows (no collectives run, so
    halo_out buffers hold garbage that would otherwise poison fp16 timing).
    """
    matsb = _shift_mats().astype(np.float16)
    ZR_A = GROUP * 4 * W
    ZR_B = GROUP * 2 * W
    w = np.arange(W, dtype=np.int64)
    zr_a = ZR_A + (w % 128)
    zr_b = ZR_B + (w % 128)
    lo_ok = s > 0 and not twin
    hi_ok = s < GROUP - 1 and not twin
    # haloA_out: rank r rows [r*4W, (r+1)*4W); field f at f*2W; side at W
    p_lo = (s - 1) * 4 * W + 0 * 2 * W + W + w if lo_ok else zr_a
    p_hi = (s + 1) * 4 * W + 0 * 2 * W + w if hi_ok else zr_a
    r_lo = (s - 1) * 4 * W + 1 * 2 * W + W + w if lo_ok else zr_a
    r_hi = (s + 1) * 4 * W + 1 * 2 * W + w if hi_ok else zr_a
    idxA = np.stack([p_lo, p_hi, r_lo, r_hi], axis=1).astype(np.int32)
    v_lo = (s - 1) * 2 * W + W + w if lo_ok else zr_b
    v_hi = (s + 1) * 2 * W + w if hi_ok else zr_b
    idxB = np.stack([v_lo, v_hi], axis=1).astype(np.int32)
    return {"matsb": matsb, "idxA": idxA, "idxB": idxB}


def make_in_maps(x, b, center, HC, W, Z):
    """Slice full inputs into per-core input maps."""
    in_maps = []
    for c in range(N_CORES):
        bi, s = divmod(c, GROUP)
        h0 = s * HC
        cen = (center[0, h0:h0 + HC, :, 0].astype(np.float32).T
               / np.float32(256.0)).copy()  # [W,HC], scaled by LAM
        m = make_const_inputs(s, HC, W)
        m.update({
            "x": np.ascontiguousarray(x[bi, h0:h0 + HC].transpose(1, 0, 2)),
            "bb": np.ascontiguousarray(b[bi, h0:h0 + HC].transpose(1, 0, 2)),
            "cen": cen,
        })
        in_maps.append(m)
    return in_maps


RUN_WALL_S = []  # wall-clock of each device dispatch (incl. axon h2d/d2h)
LAST_RESULT = None  # BassKernelResults of the most recent dispatch


def kernel(x, b, ref, center):
    """Full inputs in, full output out. ref is unused by the reference model."""
    import time as _time
    global LAST_RESULT
    B, H, W, Z = x.shape
    HC = H // GROUP
    key = (HC, W, Z)
    if key not in _CACHE:
        _CACHE[key] = build_program(HC=HC, W=W, Z=Z)
    nc = _CACHE[key]

    from concourse.bass_utils import run_bass_kernel_spmd
    in_maps = make_in_maps(np.asarray(x), np.asarray(b), np.asarray(center),
                           HC, W, Z)
    _t0 = _time.time()
    res = run_bass_kernel_spmd(nc, in_maps, core_ids=list(range(N_CORES)))
    RUN_WALL_S.append(_time.time() - _t0)
    LAST_RESULT = res
    out = np.empty((B, H, W, Z), np.float32)
    for c in range(N_CORES):
        bi, s = divmod(c, GROUP)
        out[bi, s * HC:(s + 1) * HC] = res.results[c]["xout"].transpose(
            1, 0, 2)
    return out


# revision 24
# speedup vs baseline: 40.2361x; 40.2361x over previous
"""BiCGSTAB (4 fixed iterations, 7-point stencil) on 8 Trainium2 NeuronCores.

Problem: x,b,ref: [2,256,256,256] f32, center: [1,256,256,1] f32.
reference() runs 4 BiCGSTAB iterations of A.u where A is the 7-point stencil
  S(u)[b,h,w,z] = center[h,w]*u - u[w-1] - u[w+1] - u[h-1] - u[h+1] - u[z-1] - u[z+1]
with zero Dirichlet boundaries, and global (per-batch) dot products.

Sharding: core c in 0..7 handles batch b=c//4 and H-slab [64*(c%4), 64*(c%4)+64).
Dot products become 4-rank AllReduces in groups [[0..3],[4..7]]. H-halo planes
are exchanged via AllGather within the group + indirect-DMA ghost-row gathers
(edge cores index a zeroed row range, implementing the Dirichlet boundary).

v2 design (vs f32 baseline):
- Fields r0,r,p,v,s,t stored bf16 (halved HBM traffic, 2x DVE rate); x kept
  f32. Dot products accumulate f32.
- r0 is SBUF-resident for the whole kernel (no DRAM reads for dot products).
- The ENTIRE 7-point stencil runs on the TensorEngine: W-shifts via tridiag
  matmul, H-shifts via +-Z offsets and Z-shifts via +-1 offsets on the
  flattened (h,z) moving operand, accumulated in PSUM. Two tiny DVE ops fix
  the z-wraparound columns. DVE only does the final (center*u - psum) combine
  (center*u on the Act engine).
- s = r - alpha*v is computed on the fly inside the t-pass (windows), so the
  separate P2 sweep and one full s round-trip disappear.
- Dots run partly on Pool (gpsimd) to offload DVE.
- P0 (f32 x windows) uses float32r matmuls (1 cycle/row vs 4 for plain f32).
"""
import numpy as np

import concourse.bacc as bacc
import concourse.bass as bass
import concourse.bass_isa as bass_isa
import concourse.mybir as mybir
import concourse.tile as tile

F32 = mybir.dt.float32
F16 = mybir.dt.float16
BF16 = mybir.dt.bfloat16
I32 = mybir.dt.int32

N_CORES = 8
GROUP = 4  # cores per batch group
EPS = 1e-6

KH = 8    # h planes per stencil block (fp16 passes)
KH0 = 4   # h planes per block in P0 (f32 windows)
# Stencil outputs are computed as lam*S(u) (lam folded into cen and the shift
# matrices, both exact in fp16) so v and t stay inside fp16 range; the scalar
# coefficients compensate (alpha*ILAM etc).
LAM = 1.0 / 256.0
ILAM = 256.0


def build_program(HC=64, W=256, Z=256, ITERS=4, collectives=True,
                  maxph=99, twin_reps=0):
    """Build the per-core SPMD Bass program. HC = H planes per core.

    collectives=False builds a single-core timing twin (collective_compute
    calls skipped; numerics wrong) usable for wall-clock delta timing.
    """
    assert W == 256 and Z == 256 and HC % KH == 0 and HC % KH0 == 0
    NB = HC // KH
    NB0 = HC // KH0
    RG = [list(range(GROUP)), list(range(GROUP, 2 * GROUP))]
    # halo buffer row layouts (rows = field*2W + side*W + w_global)
    ZR_A = GROUP * 4 * W   # zero-row base in haloA_out (2 fields)
    ZR_B = GROUP * 2 * W   # zero-row base in haloB/C_out (1 field)

    twin = twin_reps > 0
    assert not (twin and collectives), "twin loop cannot contain collectives"
    nc = bacc.Bacc("TRN2", target_bir_lowering=False, debug=False,
                   num_devices=N_CORES)

    if twin:
        x_in = nc.dram_tensor("xin_t", [W, HC, Z], F32)
        b_in = nc.dram_tensor("bin_t", [W, HC, Z], F32)
        x_out = nc.dram_tensor("xout_t", [W, HC, Z], F32)
        dummy_out = nc.dram_tensor("dummy_o", [1, 8], F32, kind="ExternalOutput")
    else:
        x_in = nc.dram_tensor("x", [W, HC, Z], F32, kind="ExternalInput")
        b_in = nc.dram_tensor("bb", [W, HC, Z], F32, kind="ExternalInput")
        x_out = nc.dram_tensor("xout", [W, HC, Z], F32, kind="ExternalOutput")
    cen_in = nc.dram_tensor("cen", [W, HC], F32, kind="ExternalInput")
    matsb_in = nc.dram_tensor("matsb", [128, 512], F16, kind="ExternalInput")
    idxA_in = nc.dram_tensor("idxA", [W, 4], I32, kind="ExternalInput")
    idxB_in = nc.dram_tensor("idxB", [W, 2], I32, kind="ExternalInput")

    with tile.TileContext(nc) as tc:
        with (
            tc.tile_pool(name="sb", bufs=2) as sb,
            tc.tile_pool(name="ps", bufs=8, space="PSUM") as ps,
            tc.tile_pool(name="dr", bufs=1, space="DRAM") as dr,
        ):
            _cnt = [0]

            def _nm(pfx):
                _cnt[0] += 1
                return f"{pfx}{_cnt[0]}"

            # ---- persistent DRAM intermediates
            # fields are flat [W, HC*Z + 64]: per-w-row pad of 128B breaks
            # the power-of-two row stride (HBM bank/channel conflicts) while
            # keeping each window/block DMA a single contiguous burst per
            # partition.
            FP = HC * Z + 64
            fld = {n: dr.tile([W, FP], F16, tag=n, name=f"fld_{n}")
                   for n in ("r", "p", "v", "s", "t")}
            xw = dr.tile([W, FP], F16, tag="xw", name="fld_xw")
            haloA_in = dr.tile([4 * W, Z], F16, tag="hAi")
            haloA_out = dr.tile([ZR_A + 128, Z], F16, tag="hAo")
            haloB_in = dr.tile([2 * W, Z], F16, tag="hBi")
            haloB_out = dr.tile([ZR_B + 128, Z], F16, tag="hBo")
            haloC_in = dr.tile([2 * W, Z], F32, tag="hCi")
            haloC_out = dr.tile([ZR_B + 128, Z], F32, tag="hCo")
            din = dr.tile([1, 8], F32, tag="din")
            dout = dr.tile([1, 8], F32, tag="dout")

            # ---- persistent SBUF constants
            cen_sb = []
            for wc in range(2):
                c = sb.tile([128, HC], F32, tag=f"cen{wc}", bufs=1)
                nc.sync.dma_start(out=c[:], in_=cen_in[wc * 128:(wc + 1) * 128, :])
                cen_sb.append(c)
            matsb_sb = sb.tile([128, 512], F16, tag="matsb", bufs=1)
            nc.sync.dma_start(out=matsb_sb[:], in_=matsb_in[:, :])
            idxA_sb = []
            idxB_sb = []
            for wc in range(2):
                ia = sb.tile([128, 4], I32, tag=f"idxA{wc}", bufs=1)
                nc.sync.dma_start(out=ia[:], in_=idxA_in[wc * 128:(wc + 1) * 128, :])
                idxA_sb.append(ia)
                ib = sb.tile([128, 2], I32, tag=f"idxB{wc}", bufs=1)
                nc.sync.dma_start(out=ib[:], in_=idxB_in[wc * 128:(wc + 1) * 128, :])
                idxB_sb.append(ib)
            # resident r0 (bf16), [128, HC, Z] per W-chunk
            r0_sb = [sb.tile([128, HC, Z], F16, tag=f"r0_{wc}", bufs=1,
                             name=f"r0sb{wc}") for wc in range(2)]

            # zero tails + din
            ztb = sb.tile([128, Z], F16, tag="ghb", name="ztb")
            nc.vector.memset(ztb[:], 0.0)
            nc.sync.dma_start(out=haloA_out[ZR_A:ZR_A + 128, :], in_=ztb[:])
            nc.sync.dma_start(out=haloB_out[ZR_B:ZR_B + 128, :], in_=ztb[:])
            ztf = sb.tile([128, Z], F32, tag="ghf", name="ztf")
            nc.vector.memset(ztf[:], 0.0)
            nc.sync.dma_start(out=haloC_out[ZR_B:ZR_B + 128, :], in_=ztf[:])
            z8 = sb.tile([1, 8], F32, tag="z8", bufs=1)
            nc.vector.memset(z8[:], 0.0)
            nc.sync.dma_start(out=din[:, :], in_=z8[:])

            # matrix APs
            A_b, B01_b, B10_b, I_b = (matsb_sb[:, 0:128], matsb_sb[:, 128:256],
                                      matsb_sb[:, 256:384], matsb_sb[:, 384:512])

            # ---- helpers ------------------------------------------------
            def border_order(nb):
                e = [j for j in range(nb) if 0 < j < nb - 1]
                return e + ([0] if nb == 1 else [0, nb - 1])

            def load_window(field, wc, j, tag, kh, halo_out_t, idx_t, cols,
                            dt_):
                """[128, kh+2, Z] window of planes j*kh-1 .. j*kh+kh."""
                h0 = j * kh
                nb = HC // kh
                w0 = wc * 128
                win = sb.tile([128, kh + 2, Z], dt_, tag=tag, name=_nm("win"))
                lo_g = (j == 0)
                hi_g = (j == nb - 1)
                a = 0 if lo_g else h0 - 1
                bnd = HC if hi_g else h0 + kh + 1
                po = 1 if lo_g else 0
                if len(field.shape) == 3:  # unpadded external (x_in)
                    nc.sync.dma_start(
                        out=win[:, po:po + (bnd - a), :],
                        in_=field[w0:w0 + 128, a:bnd, :])
                else:
                    wflat = win[:].rearrange("p h z -> p (h z)")
                    nc.sync.dma_start(
                        out=wflat[:, po * Z:(po + bnd - a) * Z],
                        in_=field[w0:w0 + 128, a * Z:bnd * Z])
                if lo_g:
                    nc.gpsimd.indirect_dma_start(
                        out=win[:, 0, :], out_offset=None, in_=halo_out_t[:, :],
                        in_offset=bass.IndirectOffsetOnAxis(
                            ap=idx_t[wc][:, cols[0]:cols[0] + 1], axis=0))
                if hi_g:
                    nc.gpsimd.indirect_dma_start(
                        out=win[:, kh + 1, :], out_offset=None,
                        in_=halo_out_t[:, :],
                        in_offset=bass.IndirectOffsetOnAxis(
                            ap=idx_t[wc][:, cols[1]:cols[1] + 1], axis=0))
                return win

            def stencil_tile(wins, wc, j, kh, act_wins=None, au_dt=F16):
                """vt = S(u) for chunk wc, block j, from (win0, win1).

                wins feed the PE matmuls (bf16); act_wins (default: same)
                feed the center-coefficient muls on the Act engine.
                """
                h0 = j * kh
                win = wins[wc]
                awin = (act_wins or wins)[wc]
                other = wins[1 - wc]
                wf_m = win[:].rearrange("p h z -> p (h z)")
                of_m = other[:].rearrange("p h z -> p (h z)")
                A_, I_ = A_b, I_b
                Bm = B01_b if wc == 0 else B10_b
                au = sb.tile([128, kh, Z], au_dt, tag="au", name=_nm("au"))
                for j1 in range(kh):
                    h = h0 + j1
                    nc.scalar.mul(out=au[:, j1, :], in_=awin[:, j1 + 1, :],
                                  mul=cen_sb[wc][:, h:h + 1])
                auf = au[:].rearrange("p h z -> p (h z)")
                vt = sb.tile([128, kh, Z], F16, tag=f"o{wc}", name=_nm("vt"))
                vf = vt[:].rearrange("p h z -> p (h z)")
                for q in range(kh * Z // 512):
                    c0, c1 = q * 512, (q + 1) * 512
                    ceng = nc.vector
                    pt = ps.tile([128, 512], F32, tag="pt", name=_nm("pt"))
                    nc.tensor.matmul(out=pt[:], lhsT=A_,
                                     rhs=wf_m[:, Z + c0:Z + c1],
                                     start=True, stop=False)
                    nc.tensor.matmul(out=pt[:], lhsT=I_,
                                     rhs=wf_m[:, c0:c1],
                                     start=False, stop=False)
                    nc.tensor.matmul(out=pt[:], lhsT=I_,
                                     rhs=wf_m[:, 2 * Z + c0:2 * Z + c1],
                                     start=False, stop=False)
                    nc.tensor.matmul(out=pt[:], lhsT=I_,
                                     rhs=wf_m[:, Z + c0 - 1:Z + c1 - 1],
                                     start=False, stop=False)
                    nc.tensor.matmul(out=pt[:], lhsT=I_,
                                     rhs=wf_m[:, Z + c0 + 1:Z + c1 + 1],
                                     start=False, stop=False)
                    nc.tensor.matmul(out=pt[:], lhsT=Bm,
                                     rhs=of_m[:, Z + c0:Z + c1],
                                     start=False, stop=True)
                    ceng.tensor_tensor(out=vf[:, c0:c1], in0=auf[:, c0:c1],
                                       in1=pt[:],
                                       op=mybir.AluOpType.subtract)
                # undo z-shift wraparound at z=0 / z=Z-1 (scaled by lam)
                nc.vector.scalar_tensor_tensor(
                    out=vt[:, :, 0:1], in0=win[:, 0:kh, Z - 1:Z],
                    scalar=LAM, in1=vt[:, :, 0:1],
                    op0=mybir.AluOpType.mult, op1=mybir.AluOpType.add)
                nc.vector.scalar_tensor_tensor(
                    out=vt[:, :, Z - 1:Z], in0=win[:, 2:kh + 2, 0:1],
                    scalar=LAM, in1=vt[:, :, Z - 1:Z],
                    op0=mybir.AluOpType.mult, op1=mybir.AluOpType.add)
                return vt

            def ttr(in0, in1, acc_prev, tag="accA"):
                # dot-product partial: scr = LAM*in0*in1 (discarded), acc row
                # sums. LAM keeps the fp16 products in range; the reduced
                # dots are rescaled after the AllReduce. (DVE only:
                # TensorScalarPtr is not a Pool-engine opcode.)
                scr = sb.tile([128, KH, Z], F16, tag="au", name=_nm("scr"))
                sf = scr[:].rearrange("p h z -> p (h z)")
                n = in0.free_size()
                acc = sb.tile([128, 1], F32, tag=tag + "p", bufs=4,
                              name=_nm("acc"))
                nc.vector.scalar_tensor_tensor(
                    out=sf[:, 0:n], in0=in0, scalar=LAM, in1=in1,
                    op0=mybir.AluOpType.mult, op1=mybir.AluOpType.mult,
                    accum_out=acc[:])
                if acc_prev is None:
                    return acc
                tot = sb.tile([128, 1], F32, tag=tag, bufs=4, name=_nm("accs"))
                nc.vector.tensor_add(out=tot[:], in0=acc_prev[:], in1=acc[:])
                return tot

            def finish_dot(acc, col):
                red = sb.tile([128, 1], F32, tag="dscp", bufs=8,
                              name=_nm("red"))
                nc.gpsimd.partition_all_reduce(red[:], acc[:], channels=128,
                                               reduce_op=bass_isa.ReduceOp.add)
                nc.sync.dma_start(out=din[0:1, col:col + 1], in_=red[0:1, 0:1])

            def allreduce():
                if collectives:
                    nc.gpsimd.collective_compute(
                        "AllReduce", mybir.AluOpType.add, replica_groups=RG,
                        ins=[din[:, :].opt()], outs=[dout[:, :].opt()])
                else:
                    # twin: keep dsb finite (dout is never collective-written)
                    nc.sync.dma_start(out=dout[:, :], in_=din[:, :])
                dsb = sb.tile([1, 8], F32, tag="dsb", bufs=6, name=_nm("dsb"))
                nc.sync.dma_start(out=dsb[:], in_=dout[:, :])
                return dsb

            def allgather(halo_in_t, halo_out_t, zr):
                if not collectives:
                    return
                nc.gpsimd.collective_compute(
                    "AllGather", mybir.AluOpType.bypass, replica_groups=RG,
                    ins=[halo_in_t[:, :].opt()],
                    outs=[halo_out_t[0:zr, :].opt()])

            def stage_plane(src_plane, halo_in_t, f, side, wc):
                r0_ = f * 2 * W + side * W + wc * 128
                nc.sync.dma_start(out=halo_in_t[r0_:r0_ + 128, :],
                                  in_=src_plane)

            def s_tile():
                return sb.tile([1, 1], F32, tag="dsc", bufs=16, name=_nm("sc"))

            def s_recip_eps(a_ap):
                t_ = s_tile()
                nc.vector.tensor_scalar_add(out=t_[:], in0=a_ap, scalar1=EPS)
                r_ = s_tile()
                nc.vector.reciprocal(out=r_[:], in_=t_[:])
                return r_

            def s_mul(a_ap, b_ap):
                t_ = s_tile()
                nc.vector.tensor_tensor(out=t_[:], in0=a_ap, in1=b_ap,
                                        op=mybir.AluOpType.mult)
                return t_

            def s_sub(a_ap, b_ap):
                t_ = s_tile()
                nc.vector.tensor_tensor(out=t_[:], in0=a_ap, in1=b_ap,
                                        op=mybir.AluOpType.subtract)
                return t_

            def s_neg(a_ap):
                t_ = s_tile()
                nc.vector.tensor_scalar_mul(out=t_[:], in0=a_ap, scalar1=-1.0)
                return t_

            def s_scale(a_ap, imm):
                t_ = s_tile()
                nc.vector.tensor_scalar_mul(out=t_[:], in0=a_ap, scalar1=imm)
                return t_

            def bcast(a_ap):
                b_ = sb.tile([128, 1], F32, tag="bc", bufs=8, name=_nm("bc"))
                nc.gpsimd.partition_broadcast(b_[:], a_ap, channels=128)
                return b_

            def stt(out, in0, sc, in1, eng=None):
                """out = in0*sc + in1 (sc: [128,1] AP)."""
                (eng or nc.vector).scalar_tensor_tensor(
                    out=out, in0=in0, scalar=sc, in1=in1,
                    op0=mybir.AluOpType.mult, op1=mybir.AluOpType.add)

            def load_blk(field, wc, j, tag, dt_, kh=KH):
                t_ = sb.tile([128, kh, Z], dt_, tag=tag, name=_nm("blk"))
                h0 = j * kh
                w0 = wc * 128
                if len(field.shape) == 3:  # unpadded external (x_in)
                    nc.sync.dma_start(out=t_[:],
                                      in_=field[w0:w0 + 128, h0:h0 + kh, :])
                else:
                    tf = t_[:].rearrange("p h z -> p (h z)")
                    nc.sync.dma_start(
                        out=tf[:],
                        in_=field[w0:w0 + 128, h0 * Z:(h0 + kh) * Z])
                return t_

            def store_blk(field, src, wc, j, kh=KH):
                h0 = j * kh
                w0 = wc * 128
                if len(field.shape) == 3:  # unpadded external (x_out, twin)
                    nc.sync.dma_start(out=field[w0:w0 + 128, h0:h0 + kh, :],
                                      in_=src)
                else:
                    nc.sync.dma_start(
                        out=field[w0:w0 + 128, h0 * Z:(h0 + kh) * Z],
                        in_=src.rearrange("p h z -> p (h z)"))

            if twin:
                # init big inputs so the timing loop sees normal-range fp16
                # data (uninitialized DRAM decodes to NaN/denormals)
                zi = sb.tile([128, KH, Z], F32, tag="lx", name="zinit")
                nc.vector.memset(zi[:], 0.0)
                for wc in range(2):
                    for j in range(NB):
                        store_blk(x_in, zi[:], wc, j)
                        store_blk(b_in, zi[:], wc, j)

            border = border_order(NB)
            border0 = border_order(NB0)
            # halo-producing passes: edge blocks first
            ew_order = ([0, NB - 1] if NB > 1 else [0]) + list(range(1, NB - 1))

            # ================= P0: r0 = b - S(x); rho = <r0,r0> ===========
            from contextlib import ExitStack as _ES
            _loop = _ES()
            if twin:
                _loop.enter_context(tc.For_i(0, twin_reps, 1))

            # stage x boundary planes -> haloC, gather
            for wc in range(2):
                for side, h in ((0, 0), (1, HC - 1)):
                    g = sb.tile([128, Z], F32, tag="ghf", name=_nm("gh"))
                    nc.sync.dma_start(
                        out=g[:], in_=x_in[wc * 128:wc * 128 + 128, h, :])
                    stage_plane(g[:], haloC_in, 0, side, wc)
            allgather(haloC_in, haloC_out, ZR_B)

            acc = None
            rho_ap = None
            if maxph >= 2:
                for j in border0:
                    wins = (load_window(x_in, 0, j, "w0a", KH0, haloC_out,
                                        idxB_sb, (0, 1), F32),
                            load_window(x_in, 1, j, "w1a", KH0, haloC_out,
                                        idxB_sb, (0, 1), F32))
                    bwins = []
                    for wc in range(2):
                        bw = sb.tile([128, KH0 + 2, Z], F16,
                                     tag=f"w{wc}b", name=_nm("bw"))
                        nc.scalar.copy(out=bw[:], in_=wins[wc][:])
                        bwins.append(bw)
                    for wc in range(2):
                        vt = stencil_tile(tuple(bwins), wc, j, KH0,
                                          act_wins=wins, au_dt=F32)
                        bt = load_blk(b_in, wc, j, "lx", F32, kh=KH0)
                        h0 = j * KH0
                        r0sl = r0_sb[wc][:, h0:h0 + KH0, :]
                        nc.vector.scalar_tensor_tensor(
                            out=r0sl, in0=vt[:], scalar=-ILAM, in1=bt[:],
                            op0=mybir.AluOpType.mult, op1=mybir.AluOpType.add)
                        acc = ttr(r0sl, r0sl, acc)
                        store_blk(fld["p"], r0sl, wc, j, kh=KH0)
                        store_blk(fld["r"], r0sl, wc, j, kh=KH0)
                        if j == 0:
                            stage_plane(r0_sb[wc][:, 0, :], haloA_in, 0, 0, wc)
                            stage_plane(r0_sb[wc][:, 0, :], haloA_in, 1, 0, wc)
                        if j == NB0 - 1:
                            stage_plane(r0_sb[wc][:, HC - 1, :], haloA_in,
                                        0, 1, wc)
                            stage_plane(r0_sb[wc][:, HC - 1, :], haloA_in,
                                        1, 1, wc)
                finish_dot(acc, 0)
                dsb = allreduce()
                rho_ap = s_scale(dsb[0:1, 0:1], ILAM)[:]
                allgather(haloA_in, haloA_out, ZR_A)

            for it in range(ITERS if maxph >= 3 else 0):
                last = (it == ITERS - 1)
                x_src = x_in if it == 0 else xw
                x_dst = x_out if last else xw

                # ===== P1: v = S(p); d1 = <r0, v> =====
                acc = None
                for j in border:
                    wins = (load_window(fld["p"], 0, j, "w0a", KH, haloA_out,
                                        idxA_sb, (0, 1), F16),
                            load_window(fld["p"], 1, j, "w1a", KH, haloA_out,
                                        idxA_sb, (0, 1), F16))
                    for wc in range(2):
                        vt = stencil_tile(wins, wc, j, KH)
                        h0 = j * KH
                        acc = ttr(r0_sb[wc][:, h0:h0 + KH, :], vt[:], acc)
                        store_blk(fld["v"], vt[:], wc, j)
                        if j == 0:
                            stage_plane(vt[:, 0, :], haloB_in, 0, 0, wc)
                        if j == NB - 1:
                            stage_plane(vt[:, KH - 1, :], haloB_in, 0, 1, wc)
                finish_dot(acc, 0)
                dsb = allreduce()
                allgather(haloB_in, haloB_out, ZR_B)
                d1s = s_scale(dsb[0:1, 0:1], ILAM * ILAM)
                d1_ap = d1s[:]
                alpha = s_mul(rho_ap, s_recip_eps(d1_ap)[:])
                alpha_bc = bcast(alpha[:])
                nalpha_bc = bcast(s_scale(alpha[:], -ILAM)[:])
                if maxph < 4:
                    break

                # ===== P23: s = r - alpha*v (windows, on the fly);
                #            t = S(s); <t,s>, <t,t>, <r0,t> =====
                accA = accB = accC = None
                for j in border:
                    rwins = (load_window(fld["r"], 0, j, "w0a", KH, haloA_out,
                                         idxA_sb, (2, 3), F16),
                             load_window(fld["r"], 1, j, "w1a", KH, haloA_out,
                                         idxA_sb, (2, 3), F16))
                    vwins = (load_window(fld["v"], 0, j, "w0b", KH, haloB_out,
                                         idxB_sb, (0, 1), F16),
                             load_window(fld["v"], 1, j, "w1b", KH, haloB_out,
                                         idxB_sb, (0, 1), F16))
                    for wc in range(2):
                        # s window in-place over r window
                        nc.vector.scalar_tensor_tensor(
                            out=rwins[wc][:], in0=vwins[wc][:],
                            scalar=nalpha_bc[:], in1=rwins[wc][:],
                            op0=mybir.AluOpType.mult,
                            op1=mybir.AluOpType.add)
                    for wc in range(2):
                        tt = stencil_tile(rwins, wc, j, KH)
                        s_ctr = rwins[wc][:, 1:KH + 1, :]
                        accA = ttr(tt[:], s_ctr, accA, "accA")
                        accB = ttr(tt[:], tt[:], accB, "accB")
                        h0 = j * KH
                        if not last:
                            accC = ttr(r0_sb[wc][:, h0:h0 + KH, :],
                                       tt[:], accC, "accC")
                            store_blk(fld["t"], tt[:], wc, j)
                        store_blk(fld["s"], s_ctr, wc, j)
                finish_dot(accA, 0)
                finish_dot(accB, 1)
                if not last:
                    finish_dot(accC, 2)
                dsb = allreduce()
                ts_s = s_scale(dsb[0:1, 0:1], ILAM * ILAM)
                tt_s = s_scale(dsb[0:1, 1:2], ILAM * ILAM * ILAM)
                omega = s_mul(ts_s[:], s_recip_eps(tt_s[:])[:])
                omega_bc = bcast(omega[:])
                nomega_bc = bcast(s_scale(omega[:], -ILAM)[:])
                if not last:
                    # rho' = (rho - alpha*d1) - omega*<r0,t>
                    r0t_s = s_scale(dsb[0:1, 2:3], ILAM * ILAM)
                    rho_n = s_sub(s_sub(rho_ap, s_mul(alpha[:], d1_ap)[:])[:],
                                  s_mul(omega[:], r0t_s[:])[:])
                    beta = s_mul(
                        s_mul(rho_n[:], s_recip_eps(rho_ap)[:])[:],
                        s_mul(alpha[:], s_recip_eps(omega[:])[:])[:])
                    beta_bc = bcast(beta[:])
                    rho_ap = rho_n[:]
                if maxph < 5:
                    break

                # ===== P45: x += alpha*p + omega*s;
                #       r = s - omega*t;  p = r + beta*(p - omega*v) =====
                for wc in range(2):
                    for j in (ew_order if not last else list(range(NB))):
                        xt = load_blk(x_src, wc, j, "lx", F32 if it == 0 else F16)
                        pt_ = load_blk(fld["p"], wc, j, "lp", F16)
                        st = load_blk(fld["s"], wc, j, "ls", F16)
                        x1 = sb.tile([128, KH, Z], F16, tag="x1",
                                     name=_nm("x1"))
                        stt(x1[:], pt_[:], alpha_bc[:], xt[:])
                        x2 = sb.tile([128, KH, Z], F32 if last else F16, tag="lx",
                                     name=_nm("x2"))
                        stt(x2[:], st[:], omega_bc[:], x1[:])
                        store_blk(x_dst, x2[:], wc, j)
                        if not last:
                            tt_ = load_blk(fld["t"], wc, j, "lt", F16)
                            vt_ = load_blk(fld["v"], wc, j, "lv", F16)
                            rt = sb.tile([128, KH, Z], F16, tag="ls",
                                         name=_nm("rt"))
                            stt(rt[:], tt_[:], nomega_bc[:], st[:])
                            store_blk(fld["r"], rt[:], wc, j)
                            u = sb.tile([128, KH, Z], F16, tag="lv",
                                        name=_nm("u"))
                            stt(u[:], vt_[:], nomega_bc[:], pt_[:])
                            po = sb.tile([128, KH, Z], F16, tag="lp",
                                         name=_nm("po"))
                            stt(po[:], u[:], beta_bc[:], rt[:])
                            store_blk(fld["p"], po[:], wc, j)
                            if j == 0:
                                stage_plane(po[:, 0, :], haloA_in, 0, 0, wc)
                                stage_plane(rt[:, 0, :], haloA_in, 1, 0, wc)
                            if j == NB - 1:
                                stage_plane(po[:, KH - 1, :], haloA_in,
                                            0, 1, wc)
                                stage_plane(rt[:, KH - 1, :], haloA_in,
                                            1, 1, wc)
                if last:
                    break
                allgather(haloA_in, haloA_out, ZR_A)

            _loop.close()
            if twin:
                nc.sync.dma_start(out=dummy_out[:, :], in_=z8[:])

    nc.compile()
    return nc


# ---------------------------------------------------------------------------
# host-side wrapper
# ---------------------------------------------------------------------------
_CACHE = {}


def _shift_mats():
    """[A | B01 | B10 | I] as [128, 512], scaled by LAM (exact in fp16)."""
    lam = np.float32(1.0 / 256.0)
    A = np.zeros((128, 128), np.float32)
    for i in range(127):
        A[i, i + 1] = lam
        A[i + 1, i] = lam
    B01 = np.zeros((128, 128), np.float32)
    B01[0, 127] = lam
    B10 = np.zeros((128, 128), np.float32)
    B10[127, 0] = lam
    I = lam * np.eye(128, dtype=np.float32)
    return np.concatenate([A, B01, B10, I], axis=1)


def make_const_inputs(s, HC=64, W=256, twin=False):
    """Per-core constant inputs (core's slab index s within its group).

    twin=True points every ghost at the zeroed rows (no collectives run, so
    halo_out buffers hold garbage that would otherwise poison fp16 timing).
    """
    matsb = _shift_mats().astype(np.float16)
    ZR_A = GROUP * 4 * W
    ZR_B = GROUP * 2 * W
    w = np.arange(W, dtype=np.int64)
    zr_a = ZR_A + (w % 128)
    zr_b = ZR_B + (w % 128)
    lo_ok = s > 0 and not twin
    hi_ok = s < GROUP - 1 and not twin
    # haloA_out: rank r rows [r*4W, (r+1)*4W); field f at f*2W; side at W
    p_lo = (s - 1) * 4 * W + 0 * 2 * W + W + w if lo_ok else zr_a
    p_hi = (s + 1) * 4 * W + 0 * 2 * W + w if hi_ok else zr_a
    r_lo = (s - 1) * 4 * W + 1 * 2 * W + W + w if lo_ok else zr_a
    r_hi = (s + 1) * 4 * W + 1 * 2 * W + w if hi_ok else zr_a
    idxA = np.stack([p_lo, p_hi, r_lo, r_hi], axis=1).astype(np.int32)
    v_lo = (s - 1) * 2 * W + W + w if lo_ok else zr_b
    v_hi = (s + 1) * 2 * W + w if hi_ok else zr_b
    idxB = np.stack([v_lo, v_hi], axis=1).astype(np.int32)
    return {"matsb": matsb, "idxA": idxA, "idxB": idxB}


def make_in_maps(x, b, center, HC, W, Z):
    """Slice full inputs into per-core input maps."""
    in_maps = []
    for c in range(N_CORES):
        bi, s = divmod(c, GROUP)
        h0 = s * HC
        cen = (center[0, h0:h0 + HC, :, 0].astype(np.float32).T
               / np.float32(256.0)).copy()  # [W,HC], scaled by LAM
        m = make_const_inputs(s, HC, W)
        m.update({
            "x": np.ascontiguousarray(x[bi, h0:h0 + HC].transpose(1, 0, 2)),
            "bb": np.ascontiguousarray(b[bi, h0:h0 + HC].transpose(1, 0, 2)),
            "cen": cen,
        })
        in_maps.append(m)
    return in_maps


RUN_WALL_S = []  # wall-clock of each device dispatch (incl. axon h2d/d2h)
LAST_RESULT = None  # BassKernelResults of the most recent dispatch


def kernel(x, b, ref, center):
    """Full inputs in, full output out. ref is unused by the reference model."""
    import time as _time
    global LAST_RESULT
    B, H, W, Z = x.shape
    HC = H // GROUP
    key = (HC, W, Z)
    if key not in _CACHE:
        _CACHE[key] = build_program(HC=HC, W=W, Z=Z)
    nc = _CACHE[key]

    from concourse.bass_utils import run_bass_kernel_spmd
    in_maps = make_in_maps(np.asarray(x), np.asarray(b), np.asarray(center),
                           HC, W, Z)
    _t0 = _time.time()
    res = run_bass_kernel_spmd(nc, in_maps, core_ids=list(range(N_CORES)))
    RUN_WALL_S.append(_time.time() - _t0)
    LAST_RESULT = res
    out = np.empty((B, H, W, Z), np.float32)
    for c in range(N_CORES):
        bi, s = divmod(c, GROUP)
        out[bi, s * HC:(s + 1) * HC] = res.results[c]["xout"].transpose(
            1, 0, 2)
    return out
